# revision 13
# baseline (speedup 1.0000x reference)
"""Trainium2 Bass kernel for nn_Attention_80315888435456 (location-sensitive attention).

Contract: kernel(**inputs) takes FULL unsharded inputs (B=64), shards batch over
8 NeuronCores (pure data parallel, 8 batches/core), runs one Bass/Tile kernel
via run_bass_kernel_spmd, and returns the FULL (context, weights) outputs.

Per-core device algorithm (b = local batch 0..7). T=2048 is mapped onto SBUF
partitions as t = 16*p + j (p = partition 0..127, j = 0..15), which makes the
memory / memory_after loads fully contiguous per partition (32KB / 8KB runs):
  prologue: loc[b] = conv1d(cum_attn[b]) for all b (PE matmul over K=62 im2col)
            after_query = hidden @ W_dec.T (PE)
  per b:    V     = memory_after + after_weights + query   (PSUM [128,2048])
            e     = tanh(V)                                (ACT, 2 halves)
            energ = sum_a e * W_energy[a]                  (DVE mul + reduce)
            wun   = exp(energ + maskneg), fused sum        (ACT)
            S, r  = partition-sum (PE) , reciprocal (DVE)
            wn    = wun * r          -> weights output (fp32) + f32r copy
            ctx   = sum_t wn[t] * memory[t,:]              (PE, f32r)
Large-N matmuls stream as float32r (1 cyc/row vs 4 for fp32); softmax needs no
max-subtraction: |energies| <= sum|W_energy| ~= 2.1 since tanh is bounded.
"""

import os
import sys

for _p in ("/opt/trn_rl_repo", "/root/.axon_site/_ro/trn_rl_repo"):
    if os.path.isdir(_p) and _p not in sys.path:
        sys.path.append(_p)

import numpy as np

import concourse.bass as bass
import concourse.bacc as bacc
import concourse.tile as tile
from concourse import mybir
from concourse.bass_utils import run_bass_kernel_spmd

F32 = mybir.dt.float32
F32R = mybir.dt.float32r
AF = mybir.ActivationFunctionType

B, T = 64, 2048
RNN_H, EMB, ATT, N_FILT, K = 1024, 512, 128, 32, 31
PAD = (K - 1) // 2
NCORES = 8
BPC = B // NCORES          # batches per core
NJ = T // 128              # 16 t's per partition (t = 16p + j)
TP = T + 2 * PAD           # padded conv length

_CACHE = {}
LAST_RESULTS = None        # BassKernelResults of the most recent run (for test.py)


def _build():
    nc = bacc.Bacc("TRN2", debug=False)

    # ---- DRAM I/O (per-core shards; see host prep in kernel()) ----
    # f32r-declared tensors carry plain fp32 bits; the PE rounds when streaming.
    mem_d = nc.dram_tensor("mem", [BPC, T, EMB], F32R, kind="ExternalInput")
    ma_d = nc.dram_tensor("ma", [BPC, T, ATT], F32R, kind="ExternalInput")
    cum62_d = nc.dram_tensor("cum62", [BPC, 2 * K, T], F32R, kind="ExternalInput")
    maskT_d = nc.dram_tensor("maskT", [128, BPC * NJ], F32, kind="ExternalInput")
    hT_d = nc.dram_tensor("hT", [128, 8 * BPC], F32, kind="ExternalInput")
    wdT_d = nc.dram_tensor("wdT", [128, 8 * ATT], F32, kind="ExternalInput")
    wlT_d = nc.dram_tensor("wlT", [N_FILT, ATT], F32, kind="ExternalInput")
    wc2_d = nc.dram_tensor("wc2", [2 * K, N_FILT], F32R, kind="ExternalInput")
    webc_d = nc.dram_tensor("webc", [128, T // 2], F32, kind="ExternalInput")
    eye_d = nc.dram_tensor("eye", [128, 128], F32R, kind="ExternalInput")
    ones_d = nc.dram_tensor("ones", [128, 128], F32, kind="ExternalInput")
    ctx_d = nc.dram_tensor("ctx_out", [BPC, EMB], F32, kind="ExternalOutput")
    w_d = nc.dram_tensor("w_out", [BPC, T], F32, kind="ExternalOutput")

    with tile.TileContext(nc) as tc:
        with (
            tc.tile_pool(name="static", bufs=1) as static,
            tc.tile_pool(name="memp", bufs=2) as memp,
            tc.tile_pool(name="map", bufs=2) as map_,
            tc.tile_pool(name="ep", bufs=2) as ep,
            tc.tile_pool(name="prodp", bufs=2) as prodp,
            tc.tile_pool(name="xp", bufs=1) as xp,
            tc.tile_pool(name="lop", bufs=4) as lop,
            tc.tile_pool(name="smal", bufs=2) as smal,
            tc.tile_pool(name="gath", bufs=1) as gath,
            tc.tile_pool(name="psV", bufs=2, space="PSUM") as psV,
            tc.tile_pool(name="psloc", bufs=2, space="PSUM") as psloc,
            tc.tile_pool(name="pscx", bufs=1, space="PSUM") as pscx,
            tc.tile_pool(name="psmisc", bufs=1, space="PSUM") as psmisc,
        ):
            # ---- static loads ----
            webc_sb = static.tile([128, T // 2], F32, tag="webc", name="webc_sb")
            nc.sync.dma_start(webc_sb[:], webc_d[:])
            maskT_sb = static.tile([128, BPC * NJ], F32, tag="maskT", name="maskT_sb")
            nc.sync.dma_start(maskT_sb[:], maskT_d[:])
            eye_sb = static.tile([128, 128], F32R, tag="eye", name="eye_sb")
            nc.sync.dma_start(eye_sb[:], eye_d[:])
            ones_sb = static.tile([128, 128], F32, tag="ones", name="ones_sb")
            nc.sync.dma_start(ones_sb[:], ones_d[:])
            wc2_sb = static.tile([2 * K, N_FILT], mybir.dt.bfloat16, tag="wc2", name="wc2_sb")
            nc.gpsimd.dma_start(wc2_sb[:], wc2_d[:])
            hT_sb = static.tile([128, 8 * BPC], F32, tag="hT", name="hT_sb")
            nc.sync.dma_start(hT_sb[:], hT_d[:])
            wdT_sb = static.tile([128, 8 * ATT], F32, tag="wdT", name="wdT_sb")
            nc.sync.dma_start(wdT_sb[:], wdT_d[:])

            # per-batch [33,128] rhs for the aw matmul: rows 0..31 = W_loc^T,
            # row 32 = after_query[b]
            wlq = []
            for b in range(BPC):
                t = static.tile([N_FILT + 2, ATT], mybir.dt.bfloat16, tag=f"wlq{b}", name=f"wlq{b}")
                nc.gpsimd.dma_start(t[0:N_FILT, :], wlT_d[:])
                wlq.append(t)

            # ---- after_query = hidden @ W_dec.T for all 8 batches ----
            qps = pscx.tile([BPC, ATT], F32, tag="cx", name="qps")
            for hc in range(8):
                nc.tensor.matmul(
                    qps[:],
                    hT_sb[:, hc * BPC:(hc + 1) * BPC],
                    wdT_sb[:, hc * ATT:(hc + 1) * ATT],
                    start=(hc == 0),
                    stop=(hc == 7),
                )
            q_sb = gath.tile([BPC, ATT], F32, tag="q_sb", name="q_sb")
            nc.scalar.copy(q_sb[:], qps[:])
            q_hi = gath.tile([BPC, ATT], mybir.dt.bfloat16, tag="q_hi", name="q_hi")
            nc.vector.tensor_copy(q_hi[:], q_sb[:])
            q_lo = gath.tile([BPC, ATT], F32, tag="q_lo", name="q_lo")
            nc.vector.tensor_sub(q_lo[:], q_sb[:], q_hi[:])
            for b in range(BPC):
                nc.gpsimd.dma_start(wlq[b][N_FILT:N_FILT + 1, :], q_hi[b:b + 1, :])
                nc.gpsimd.dma_start(wlq[b][N_FILT + 1:N_FILT + 2, :], q_lo[b:b + 1, :])

            # ---- software pipeline: x62 all upfront; conv / ma / mem
            # prefetched two batches ahead of consumption ----
            ones_flat = ones_d[:].rearrange("a b -> (a b)")
            x62s = []
            for b in range(BPC):
                x62 = xp.tile([2 * K, T], mybir.dt.bfloat16, tag="x62",
                              name=f"x62_{b}", bufs=BPC)
                nc.gpsimd.dma_start(x62[:], cum62_d[b])
                x62s.append(x62)

            locones = {}
            ma_t = {}
            mem_t = {}

            def fetch(b):
                if b >= BPC:
                    return
                ma_t[b] = map_.tile([128, T], F32R, tag="ma", name=f"ma_{b}")
                nc.gpsimd.dma_start(ma_t[b][:], ma_d[b])
                mem_t[b] = memp.tile([128, NJ * EMB], F32R, tag="mem",
                                     name=f"mem_{b}")
                nc.scalar.dma_start(mem_t[b][:], mem_d[b])

            def conv_chain(b):
                if b >= BPC:
                    return
                lo = lop.tile([N_FILT + 2, T], mybir.dt.bfloat16, tag="lo", name=f"lo{b}", bufs=4)
                nc.gpsimd.dma_start(lo[N_FILT:N_FILT + 2, :], ones_flat[0:2 * T])
                for g in range(4):
                    locps = psloc.tile([N_FILT, 512], F32, tag="locps", name="locps")
                    nc.tensor.matmul(
                        locps[:], wc2_sb[:], x62s[b][:, g * 512:(g + 1) * 512],
                        start=True, stop=True,
                    )
                    nc.scalar.copy(lo[0:N_FILT, g * 512:(g + 1) * 512], locps[:])
                locones[b] = lo

            for b in (0, 1):
                fetch(b)
                conv_chain(b)

            wgath = gath.tile([128, BPC * NJ], F32, tag="wgath", name="wgath")

            for b in range(BPC):
                # ---- V = ma + after_weights + query, two pipelined halves.
                energ = smal.tile([128, NJ], F32, tag="energ", name="energ")
                HT = T // 2
                for h in range(2):
                    Vh = psV.tile([128, HT], F32, tag="V", name="Vh")
                    for gg in range(2):  # identity-add of memory_after
                        g = 2 * h + gg
                        nc.tensor.matmul(
                            Vh[:, gg * 512:(gg + 1) * 512],
                            eye_sb[:],
                            ma_t[b][:, g * 512:(g + 1) * 512],
                            start=True, stop=False,
                        )
                    for jj in range(NJ // 2):  # after_weights + query
                        j = h * (NJ // 2) + jj
                        nc.tensor.matmul(
                            Vh[:, jj * 128:(jj + 1) * 128],
                            locones[b][:, j:T:NJ],
                            wlq[b][:],
                            start=False, stop=(jj % 4 == 3),
                        )
                    e_t = ep.tile([128, HT], F32, tag=f"e{h}", name=f"e_t{h}", bufs=1)
                    nc.scalar.activation(e_t[:], Vh[:], AF.Tanh)
                    prod = prodp.tile([128, HT], F32, tag="prod", name="prod", bufs=1)
                    nc.vector.tensor_mul(prod[:], e_t[:], webc_sb[:])
                    nc.vector.reduce_sum(
                        energ[:, h * (NJ // 2):(h + 1) * (NJ // 2)],
                        prod.rearrange("p (j a) -> p j a", j=NJ // 2),
                        axis=mybir.AxisListType.X,
                    )

                # ---- softmax (no max-sub; mask = -1e30) ----
                em = smal.tile([128, NJ], F32, tag="em", name="em")
                nc.vector.tensor_add(em[:], energ[:], maskT_sb[:, b * NJ:(b + 1) * NJ])
                wun = smal.tile([128, NJ], F32, tag="wun", name="wun")
                wsum = smal.tile([128, 1], F32, tag="wsum", name="wsum")
                nc.scalar.activation(wun[:], em[:], AF.Exp, accum_out=wsum[:])

                misc = psmisc.tile([128, 512], F32, tag="misc", name="misc")
                # S = sum over partitions of wsum
                nc.tensor.matmul(misc[0:1, 0:1], ones_sb[:, 0:1], wsum[:],
                                 start=True, stop=True)
                r_t = smal.tile([1, 1], F32, tag="r_t", name="r_t")
                nc.vector.reciprocal(r_t[:], misc[0:1, 0:1])
                # broadcast r to 128 partitions via rank-1 matmul
                nc.tensor.matmul(misc[:, 1:2], ones_sb[0:1, :], r_t[:],
                                 start=True, stop=True)
                # normalized weights: fp32 into the output gather + f32r copy
                # for the context matmul lhsT
                wn = wgath[:, b * NJ:(b + 1) * NJ]
                nc.vector.tensor_scalar_mul(wn, wun[:], misc[:, 1:2])
                wnr = smal.tile([128, NJ], F32R, tag="wnr", name="wnr")
                nc.vector.tensor_copy(wnr[:], wn)

                # ---- context = sum_t wn[t] * memory[t, :] ----
                ctxps = pscx.tile([1, EMB], F32, tag="cx", name="ctxps")
                for j in range(NJ):
                    nc.tensor.matmul(
                        ctxps[:],
                        wnr[:, j:j + 1],
                        mem_t[b][:, j * EMB:(j + 1) * EMB],
                        start=(j == 0), stop=(j == NJ - 1),
                    )
                ctxst = smal.tile([1, EMB], F32, tag="ctxst", name="ctxst")
                nc.vector.tensor_copy(ctxst[:], ctxps[:])
                nc.sync.dma_start(ctx_d[b].unsqueeze(0), ctxst[:])

                fetch(b + 2)
                conv_chain(b + 2)

            # ---- store outputs ----
            nc.sync.dma_start(
                w_d[:].rearrange("b (p j) -> p b j", p=128),
                wgath.rearrange("p (b j) -> p b j", b=BPC),
            )

    nc.compile()
    return nc


def _prep_core(inputs, core):
    lo, hi = core * BPC, (core + 1) * BPC
    mem = np.ascontiguousarray(inputs["memory"][lo:hi], dtype=np.float32)
    ma = np.ascontiguousarray(inputs["memory_after"][lo:hi], dtype=np.float32)
    cum = np.zeros((BPC, 2, TP), np.float32)
    cum[:, :, PAD:PAD + T] = inputs["cumulative_attention_weights"][lo:hi]
    idx = np.arange(K)[:, None] + np.arange(T)[None, :]        # [K, T]
    cum62 = cum[:, :, idx].reshape(BPC, 2 * K, T)              # [b,(i,k),t]
    mask = np.asarray(inputs["mask"][lo:hi])
    maskT = np.where(mask, np.float32(-1e30), np.float32(0.0)).astype(np.float32)
    # [p, b*16+j] with t = 16p + j
    maskT = maskT.reshape(BPC, 128, NJ).transpose(1, 0, 2).reshape(128, BPC * NJ)
    hid = np.asarray(inputs["hidden_atten_state"][lo:hi], dtype=np.float32)
    hT = hid.reshape(BPC, 8, 128).transpose(2, 1, 0).reshape(128, 8 * BPC)
    wd = np.asarray(inputs["W_dec"], dtype=np.float32)  # [ATT, RNN_H]
    wdT = wd.reshape(ATT, 8, 128).transpose(2, 1, 0).reshape(128, 8 * ATT)
    wlT = np.ascontiguousarray(np.asarray(inputs["W_loc"], np.float32).T)  # [32,128]
    wc2 = np.ascontiguousarray(
        np.asarray(inputs["W_conv"], np.float32).transpose(1, 2, 0).reshape(2 * K, N_FILT)
    )
    we = np.asarray(inputs["W_energy"], np.float32)[0]  # [128]
    webc = np.ascontiguousarray(np.broadcast_to(np.tile(we, NJ // 2), (128, T // 2)))
    eye = np.eye(128, dtype=np.float32)
    ones = np.ones((128, 128), np.float32)
    return {
        "mem": mem, "ma": ma, "cum62": np.ascontiguousarray(cum62), "maskT": np.ascontiguousarray(maskT),
        "hT": np.ascontiguousarray(hT), "wdT": np.ascontiguousarray(wdT),
        "wlT": wlT, "wc2": wc2, "webc": webc, "eye": eye, "ones": ones,
    }


def kernel(**inputs):
    global LAST_RESULTS
    if "nc" not in _CACHE:
        _CACHE["nc"] = _build()
    nc = _CACHE["nc"]
    in_maps = [_prep_core(inputs, c) for c in range(NCORES)]
    res = run_bass_kernel_spmd(nc, in_maps, list(range(NCORES)))
    LAST_RESULTS = res
    context = np.concatenate([res.results[c]["ctx_out"] for c in range(NCORES)], axis=0)
    weights = np.concatenate([res.results[c]["w_out"] for c in range(NCORES)], axis=0)
    return context.astype(np.float32), weights.astype(np.float32)
